# revision 1
# baseline (speedup 1.0000x reference)
"""Trainium2 Bass kernel for nn_LongShortAttention (sparse local+global attention).

Sharding: 8 NeuronCores; core c owns batch c//4, tokens [(c%4)*1024, +1024).
Each core computes Q/KV projections for its tokens (KV with a 128-token left
halo), the windowed local branch, the compressed global branch (own 64
segments, then a tiny AllGather of the LayerNormed compressed KV across the 4
cores of its batch), softmax over [global | local] keys, attention output, and
its token rows of the final output projection.  Matmuls run in float32r.
The program is identical on all 8 cores; per-core behavior (halo validity,
causal masks, token offsets) enters only through input data.

LayerNorm statistics (mean, mean-of-squares) and the segment-compress logits z
are computed as tiny PE matmuls on the d-major kv, bounced through DRAM into
token-major columns, so the per-tile LN apply is a single tensor_scalar op.
Softmax normalization 1/Z is computed as exp(-ln(Z)) on the scalar engine
(vector reciprocal is 8 cyc/elem) and broadcast across partitions via a DRAM
round-trip.

Self-contained: numpy + concourse imports only; all shapes hardcoded.
"""
import contextlib

import numpy as np

import concourse.bass as bass
import concourse.mybir as mybir
import concourse.tile as tile
from concourse import bacc
from concourse.bass_utils import run_bass_kernel_spmd

A = mybir.AluOpType
AF = mybir.ActivationFunctionType
F32 = mybir.dt.float32
F32R = mybir.dt.float32r

B, N, DIM, H, D = 2, 4096, 1024, 16, 64
W, S, R = 128, 16, 1
EPS = 1e-5
SCALE = D ** -0.5
NC = 8
TOK = 1024
HALO = 128
TOKH = 1152
NT = 9                      # token tiles incl halo (tt=0 is halo)
NSEG = TOKH // S            # 72 segments incl halo
P = 128


def _f32(x):
    return np.ascontiguousarray(x, dtype=np.float32)


def build_program(nontrivial_ln_l=False, nontrivial_ln_g=False,
                  nonzero_bq=False, nonzero_bkv=False, nonzero_bo=False):
    nc = bacc.Bacc(None, target_bir_lowering=False, debug=False)

    xt = nc.declare_dram_parameter("xt", [DIM, TOKH], F32R, isOutput=False)
    wq = nc.declare_dram_parameter("wq", [DIM, DIM], F32R, isOutput=False)
    wkv = nc.declare_dram_parameter("wkv", [DIM, DIM], F32R, isOutput=False)
    wo = nc.declare_dram_parameter("wo", [DIM, DIM], F32R, isOutput=False)
    ident_d = nc.declare_dram_parameter("ident", [P, P], F32R, isOutput=False)
    seg16_d = nc.declare_dram_parameter("seg16", [P, 8], F32R, isOutput=False)
    tri_d = nc.declare_dram_parameter("tri", [P, P], F32R, isOutput=False)
    halo_d = nc.declare_dram_parameter("halom", [P, P], F32R, isOutput=False)
    gmask_d = nc.declare_dram_parameter("gmask", [P, 2, 2, 512], F32R, isOutput=False)
    stats_d = nc.declare_dram_parameter("stats_lhsT", [P, 4], F32R, isOutput=False)
    ones_d = nc.declare_dram_parameter("onesbig", [P, NT * 16 * 65], F32R, isOutput=False)
    if nontrivial_ln_l:
        lgl_d = nc.declare_dram_parameter("ln_l_gb", [P, 2, 64], F32R, isOutput=False)
    if nontrivial_ln_g:
        lgg_d = nc.declare_dram_parameter("ln_g_gb", [P, 2, 64], F32R, isOutput=False)
    if nonzero_bq:
        bq_d = nc.declare_dram_parameter("bqs", [P, 8], F32, isOutput=False)
    if nonzero_bkv:
        bkv_d = nc.declare_dram_parameter("bkvs", [P, 8], F32, isOutput=False)
    if nonzero_bo:
        bo_d = nc.declare_dram_parameter("bod", [1, DIM], F32R, isOutput=False)
        ones128_d = nc.declare_dram_parameter("ones128", [1, P], F32R, isOutput=False)
    out_d = nc.declare_dram_parameter("out", [8, P, DIM], F32, isOutput=True)

    with tile.TileContext(nc) as tc:
        stack = contextlib.ExitStack()
        with stack:
            dram = stack.enter_context(tc.tile_pool(name="dram", bufs=1, space="DRAM"))
            consts = stack.enter_context(tc.tile_pool(name="consts", bufs=1))

            pool_qT = tc.alloc_tile_pool(name="p_qT", bufs=1)
            pool_kvT = tc.alloc_tile_pool(name="p_kvT", bufs=1, side="right")
            pool_mid = tc.alloc_tile_pool(name="p_mid", bufs=1, side="right")

            ident = consts.tile([P, P], F32R)
            nc.sync.dma_start(out=ident[:], in_=ident_d[:])
            seg16 = consts.tile([P, 8], F32R)
            nc.sync.dma_start(out=seg16[:], in_=seg16_d[:])
            tri = consts.tile([P, P], F32R)
            nc.sync.dma_start(out=tri[:], in_=tri_d[:])
            halom = consts.tile([P, P], F32R)
            nc.sync.dma_start(out=halom[:], in_=halo_d[:])
            gmask = consts.tile([P, 2, 2, 512], F32R)
            nc.sync.dma_start(out=gmask[:], in_=gmask_d[:])
            stats_lhsT = consts.tile([P, 4], F32R)
            nc.sync.dma_start(out=stats_lhsT[:], in_=stats_d[:])
            eps_t = consts.tile([P, 1], F32)
            nc.vector.memset(eps_t[:], EPS)
            if nontrivial_ln_l:
                lgl = consts.tile([P, 2, 64], F32R)
                nc.sync.dma_start(out=lgl[:], in_=lgl_d[:])
            if nontrivial_ln_g:
                lgg = consts.tile([P, 2, 64], F32R)
                nc.sync.dma_start(out=lgg[:], in_=lgg_d[:])
            if nonzero_bq:
                bqs = consts.tile([P, 8], F32)
                nc.sync.dma_start(out=bqs[:], in_=bq_d[:])
            if nonzero_bkv:
                bkvs = consts.tile([P, 8], F32)
                nc.sync.dma_start(out=bkvs[:], in_=bkv_d[:])
            if nonzero_bo:
                bod = consts.tile([1, DIM], F32R)
                nc.sync.dma_start(out=bod[:], in_=bo_d[:])
                ones128 = consts.tile([1, P], F32R)
                nc.sync.dma_start(out=ones128[:], in_=ones128_d[:])

            qT = pool_qT.tile([P, 8, TOK], F32R)      # [dim-in-m, m, tok]
            kvT = pool_kvT.tile([P, 8, TOKH], F32R)   # [dim-in-m, m, tok+halo]

            # ------------- Phase B: projections + stats matmuls -------------
            sdram_kv = dram.tile([4, 8, TOKH], F32)   # rows mu_h0, z_h0, mu_h1, z_h1
            sdram_sq = dram.tile([4, 8, TOKH], F32)   # rows e2_h0, -, e2_h1, -
            with tc.tile_pool(name="xw", bufs=8) as xw_pool, \
                 tc.tile_pool(name="wld", bufs=8) as wld_pool, \
                 tc.tile_pool(name="sq", bufs=2) as sq_pool, \
                 tc.tile_pool(name="zstage", bufs=1) as zstage_pool, \
                 tc.tile_pool(name="pproj", bufs=3, space="PSUM") as pproj, \
                 tc.tile_pool(name="pz", bufs=1, space="PSUM") as pz:

                xt_k = []
                for k in range(8):
                    xk = xw_pool.tile([P, TOKH], F32R, tag="xk")
                    nc.sync.dma_start(out=xk[:], in_=xt[k * P:(k + 1) * P, :])
                    xt_k.append(xk)

                for wname, wdram in (("kv", wkv), ("q", wq)):
                    w_k = []
                    for k in range(8):
                        wk2 = wld_pool.tile([P, DIM], F32R, tag="wmat")
                        nc.sync.dma_start(out=wk2[:], in_=wdram[k * P:(k + 1) * P, :])
                        w_k.append(wk2)
                    for m in range(8):
                        if wname == "q":
                            for nt2 in range(2):
                                ps = pproj.tile([P, 512], F32, tag="proj")
                                for k in range(8):
                                    nc.tensor.matmul(
                                        ps[:],
                                        w_k[k][:, m * P:(m + 1) * P],
                                        xt_k[k][:, HALO + nt2 * 512:HALO + nt2 * 512 + 512],
                                        start=(k == 0), stop=(k == 7))
                                dst = qT[:, m, nt2 * 512:(nt2 + 1) * 512]
                                if nonzero_bq:
                                    nc.scalar.activation(dst, ps[:], AF.Identity,
                                                         bias=bqs[:, m:m + 1],
                                                         scale=SCALE)
                                else:
                                    nc.scalar.activation(dst, ps[:], AF.Copy,
                                                         scale=SCALE)
                        else:
                            for nt3 in range(3):
                                ps = pproj.tile([P, 512], F32, tag="proj")
                                for k in range(8):
                                    nc.tensor.matmul(
                                        ps[:, :384],
                                        w_k[k][:, m * P:(m + 1) * P],
                                        xt_k[k][:, nt3 * 384:nt3 * 384 + 384],
                                        start=(k == 0), stop=(k == 7))
                                dst = kvT[:, m, nt3 * 384:(nt3 + 1) * 384]
                                if nonzero_bkv:
                                    nc.scalar.activation(dst, ps[:, :384],
                                                         AF.Identity,
                                                         bias=bkvs[:, m:m + 1],
                                                         scale=1.0)
                                else:
                                    nc.scalar.activation(dst, ps[:, :384], AF.Copy,
                                                         scale=1.0)

                # stats rows: [mu_h0, z_h0, mu_h1, z_h1] @ kvT ; e2 via kv^2
                for m in range(8):
                    sqt = sq_pool.tile([P, TOKH], F32R, tag="sqt")
                    nc.vector.tensor_tensor(out=sqt[:], in0=kvT[:, m, :],
                                            in1=kvT[:, m, :], op=A.mult)
                    psz = pz.tile([4, 3, 512], F32, tag="zp")
                    psz2 = pz.tile([4, 3, 512], F32, tag="zp")
                    for nt3 in range(3):
                        nc.tensor.matmul(
                            psz[:, nt3, :384], stats_lhsT[:],
                            kvT[:, m, nt3 * 384:nt3 * 384 + 384],
                            start=True, stop=True)
                        nc.tensor.matmul(
                            psz2[:, nt3, :384], stats_lhsT[:],
                            sqt[:, nt3 * 384:nt3 * 384 + 384],
                            start=True, stop=True)
                    zst = zstage_pool.tile([4, TOKH], F32, tag="zst")
                    nc.scalar.activation(
                        zst[:].rearrange("p (a b) -> p a b", a=3, b=384),
                        psz[:, :, :384], AF.Copy)
                    nc.sync.dma_start(out=sdram_kv[:, m, :], in_=zst[:])
                    zst2 = zstage_pool.tile([4, TOKH], F32, tag="zst2")
                    nc.scalar.activation(
                        zst2[:].rearrange("p (a b) -> p a b", a=3, b=384),
                        psz2[:, :, :384], AF.Copy)
                    nc.sync.dma_start(out=sdram_sq[:, m, :], in_=zst2[:])

            # ------------- Phase C: columns (rstd, bcol) + z softmax ----------
            pcol = pool_mid.tile([P, NT, 16], F32)
            rstd_col = pool_mid.tile([P, NT, 16], F32)
            bcol_col = pool_mid.tile([P, NT, 16], F32)
            with tc.tile_pool(name="zseg", bufs=1) as zseg_pool:
                # segment softmax of z
                zseg = zseg_pool.tile([NSEG, 16, S], F32)
                for par in range(2):
                    nc.gpsimd.dma_start(
                        out=zseg[:, par::2, :],
                        in_=sdram_kv[2 * par + 1].rearrange("m (g s) -> g m s", s=S))
                ez = zseg_pool.tile([NSEG, 16, S], F32)
                nc.scalar.activation(ez[:], zseg[:], AF.Exp)
                sz = zseg_pool.tile([NSEG, 16], F32)
                nc.vector.reduce_sum(sz[:], ez[:], axis=mybir.AxisListType.X)
                lsz = zseg_pool.tile([NSEG, 16], F32)
                nc.scalar.activation(lsz[:], sz[:], AF.Ln)
                rz = zseg_pool.tile([NSEG, 16], F32)
                nc.scalar.activation(rz[:], lsz[:], AF.Exp, scale=-1.0)
                pseg = zseg_pool.tile([NSEG, S, 16], F32)
                for h in range(16):
                    nc.vector.tensor_scalar_mul(
                        pseg[:, :, h], ez[:, h, :], rz[:, h:h + 1])
                pdram = dram.tile([NSEG, S, 16], F32)
                nc.sync.dma_start(out=pdram[:], in_=pseg[:])
                nc.gpsimd.dma_start(
                    out=pcol[:],
                    in_=pdram[:].rearrange("(t g) s h -> (g s) t h", g=8))
                # mu / e2 in segment-major (seg, h, s) -> rstd, bcol -> columns
                mseg = zseg_pool.tile([NSEG, 16, S], F32)
                eseg = zseg_pool.tile([NSEG, 16, S], F32)
                for par in range(2):
                    nc.gpsimd.dma_start(
                        out=mseg[:, par::2, :],
                        in_=sdram_kv[2 * par].rearrange("m (g s) -> g m s", s=S))
                    nc.gpsimd.dma_start(
                        out=eseg[:, par::2, :],
                        in_=sdram_sq[2 * par].rearrange("m (g s) -> g m s", s=S))
                var = zseg_pool.tile([NSEG, 16, S], F32)
                nc.vector.tensor_tensor(out=var[:], in0=mseg[:], in1=mseg[:],
                                        op=A.mult)
                nc.vector.tensor_tensor(out=var[:], in0=eseg[:], in1=var[:],
                                        op=A.subtract)
                sdv = zseg_pool.tile([NSEG, 16, S], F32)
                nc.scalar.activation(sdv[:], var[:], AF.Sqrt, bias=eps_t[:NSEG])
                rs_sh = zseg_pool.tile([NSEG, S, 16], F32)
                bc_sh = zseg_pool.tile([NSEG, S, 16], F32)
                for h in range(16):
                    nc.vector.reciprocal(rs_sh[:, :, h], sdv[:, h, :])
                    nc.vector.scalar_tensor_tensor(
                        out=bc_sh[:, :, h], in0=mseg[:, h, :], scalar=-1.0,
                        in1=rs_sh[:, :, h], op0=A.mult, op1=A.mult)
                rs_dram = dram.tile([NSEG, S, 16], F32)
                bc_dram = dram.tile([NSEG, S, 16], F32)
                nc.sync.dma_start(out=rs_dram[:], in_=rs_sh[:])
                nc.sync.dma_start(out=bc_dram[:], in_=bc_sh[:])
                nc.gpsimd.dma_start(
                    out=rstd_col[:],
                    in_=rs_dram[:].rearrange("(t g) s h -> (g s) t h", g=8))
                nc.gpsimd.dma_start(
                    out=bcol_col[:],
                    in_=bc_dram[:].rearrange("(t g) s h -> (g s) t h", g=8))

            # ------------- Phase D: transpose kv, LN, compress, gather --------
            pool_attn = tc.alloc_tile_pool(name="p_attn", bufs=1)
            v_ln = pool_attn.tile([P, NT, 16, 65], F32R)
            gv = pool_attn.tile([P, 2, 16, 65], F32R)
            gkvT = pool_attn.tile([P, 2, 8, P], F32R)
            nc.sync.dma_start(out=v_ln[:], in_=ones_d[:].rearrange(
                "p (a b c) -> p a b c", a=NT, b=16, c=65))
            nc.sync.dma_start(out=gv[:], in_=ones_d[:, :2 * 16 * 65].rearrange(
                "p (a b c) -> p a b c", a=2, b=16, c=65))

            gkvr_dram = dram.tile([8, 8, 16, 64], F32)     # [tt-1, g, head, d]
            with tc.tile_pool(name="ptok", bufs=4, space="PSUM") as ptokp, \
                 tc.tile_pool(name="pg", bufs=2, space="PSUM") as pgp, \
                 tc.tile_pool(name="gst", bufs=2) as gst_pool, \
                 tc.tile_pool(name="wscr", bufs=4) as wscrp:
                for tt in range(NT):
                    if tt >= 1:
                        pgA = pgp.tile([8, 8, 64], F32, tag="pga")
                        pgB = pgp.tile([8, 8, 64], F32, tag="pgb")
                    for m in range(8):
                        ptok = ptokp.tile([P, P], F32R, tag="ptok")
                        nc.tensor.transpose(
                            ptok[:], kvT[:, m, tt * P:(tt + 1) * P], ident[:])
                        for par in range(2):
                            h = 2 * m + par
                            hs = ptok[:, par * 64:(par + 1) * 64]
                            with nc.allow_low_precision(reason="f32r store"):
                                nc.vector.tensor_scalar(
                                    out=v_ln[:, tt, h, 0:64], in0=hs,
                                    scalar1=rstd_col[:, tt, h:h + 1],
                                    scalar2=bcol_col[:, tt, h:h + 1],
                                    op0=A.mult, op1=A.add)
                                if nontrivial_ln_l:
                                    nc.vector.scalar_tensor_tensor(
                                        out=v_ln[:, tt, h, 0:64],
                                        in0=v_ln[:, tt, h, 0:64], scalar=1.0,
                                        in1=lgl[:, 0, :], op0=A.mult, op1=A.mult)
                                    nc.vector.tensor_tensor(
                                        out=v_ln[:, tt, h, 0:64],
                                        in0=v_ln[:, tt, h, 0:64],
                                        in1=lgl[:, 1, :], op=A.add)
                            if tt >= 1:
                                wscr = wscrp.tile([P, 64], F32R, tag="wscr")
                                with nc.allow_low_precision(reason="f32r store"):
                                    nc.vector.tensor_scalar_mul(
                                        wscr[:], hs, pcol[:, tt, h:h + 1])
                                pgx = pgA if h < 8 else pgB
                                nc.tensor.matmul(pgx[:, h % 8, :], seg16[:],
                                                 wscr[:], start=True, stop=True)
                                if h in (7, 15):
                                    gst = gst_pool.tile([8, 8, 64], F32, tag="gst")
                                    nc.scalar.activation(
                                        gst[:], (pgA if h == 7 else pgB)[:],
                                        AF.Copy)
                                    nc.sync.dma_start(
                                        out=gkvr_dram[tt - 1, :, (h // 8) * 8:
                                                      (h // 8) * 8 + 8, :],
                                        in_=gst[:])

            # own-seg LN of compressed kv, AllGather across batch group
            with tc.tile_pool(name="gln", bufs=1) as gln_pool:
                glnin = gln_pool.tile([64, 16, 64], F32)
                nc.sync.dma_start(out=glnin[:],
                                  in_=gkvr_dram[:].rearrange("t g h d -> (t g) h d"))
                glnout = gln_pool.tile([64, 16, 64], F32)
                st2 = gln_pool.tile([64, 16, 6], F32)
                for h in range(16):
                    nc.vector.bn_stats(out=st2[:, h, :], in_=glnin[:, h, :])
                mv2 = gln_pool.tile([64, 16, 2], F32)
                for h in range(16):
                    nc.vector.bn_aggr(out=mv2[:, h, :], in_=st2[:, h, :])
                sd2 = gln_pool.tile([64, 16], F32)
                nc.scalar.activation(sd2[:], mv2[:, :, 1], AF.Sqrt,
                                     bias=eps_t[:64])
                rstd2 = gln_pool.tile([64, 16], F32)
                nc.vector.reciprocal(rstd2[:], sd2[:])
                bcol2 = gln_pool.tile([64, 16], F32)
                nc.vector.scalar_tensor_tensor(
                    out=bcol2[:], in0=mv2[:, :, 0], scalar=-1.0, in1=rstd2[:],
                    op0=A.mult, op1=A.mult)
                for h in range(16):
                    nc.vector.tensor_scalar(
                        out=glnout[:, h, :], in0=glnin[:, h, :],
                        scalar1=rstd2[:, h:h + 1], scalar2=bcol2[:, h:h + 1],
                        op0=A.mult, op1=A.add)
                    if nontrivial_ln_g:
                        nc.vector.scalar_tensor_tensor(
                            out=glnout[:, h, :], in0=glnout[:, h, :], scalar=1.0,
                            in1=lgg[:64, 0, :], op0=A.mult, op1=A.mult)
                        nc.vector.tensor_tensor(
                            out=glnout[:, h, :], in0=glnout[:, h, :],
                            in1=lgg[:64, 1, :], op=A.add)

                cc_in = dram.tile([16, 64, 64], F32)
                nc.sync.dma_start(out=cc_in[:].transpose([1, 0, 2]), in_=glnout[:])
                cc_out = dram.tile([4, 16, 64, 64], F32)
                nc.gpsimd.collective_compute(
                    "AllGather", A.bypass,
                    replica_groups=[[0, 1, 2, 3], [4, 5, 6, 7]],
                    ins=[cc_in.opt()], outs=[cc_out.opt()])
                for b in range(2):
                    for cg in range(2):
                        nc.sync.dma_start(
                            out=gv[64 * cg:64 * cg + 64, b, :, 0:64].bitcast(F32),
                            in_=cc_out[2 * b + cg].transpose([1, 0, 2]))

            with tc.tile_pool(name="pgt", bufs=2, space="PSUM") as pgt:
                for b in range(2):
                    for mg in range(2):
                        pst = pgt.tile([64, 4, P], F32R, tag="pgt")
                        pst2 = pgt.tile([64, 4, P], F32R, tag="pgt2")
                        for j in range(4):
                            m = 4 * mg + j
                            nc.tensor.transpose(pst[:, j, :],
                                                gv[:, b, 2 * m, 0:64], ident[:])
                            nc.tensor.transpose(pst2[:, j, :],
                                                gv[:, b, 2 * m + 1, 0:64], ident[:])
                        nc.scalar.activation(
                            gkvT[0:64, b, 4 * mg:4 * mg + 4, :], pst[:], AF.Copy)
                        nc.scalar.activation(
                            gkvT[64:128, b, 4 * mg:4 * mg + 4, :], pst2[:], AF.Copy)

            pool_mid.release()
            pool_kvT.release()

            # ---------------- Phase E: attention per head-pair ----------------
            pool_out = tc.alloc_tile_pool(name="p_out", bufs=1, side="right")
            attnT = pool_out.tile([P, 8, TOK], F32R)
            zr_dram = dram.tile([16, 2, 512], F32)
            with tc.tile_pool(name="lkvT", bufs=2) as lkvp, \
                 tc.tile_pool(name="expl", bufs=1) as explp, \
                 tc.tile_pool(name="expg", bufs=1) as expgp, \
                 tc.tile_pool(name="ptr", bufs=1, space="PSUM") as ptr, \
                 tc.tile_pool(name="plsim", bufs=1, space="PSUM") as plsim, \
                 tc.tile_pool(name="pgsim", bufs=1, space="PSUM") as pgsim, \
                 tc.tile_pool(name="pav", bufs=2, space="PSUM") as pav, \
                 tc.tile_pool(name="evs", bufs=2) as evs:
                for m in range(8):
                    lkvT = lkvp.tile([P, NT, P], F32R, tag="lkvT")
                    for ug in range(3):
                        nu = 4 if ug < 2 else 1
                        ptt = ptr.tile([64, 4, P], F32R, tag="ptr")
                        ptt2 = ptr.tile([64, 4, P], F32R, tag="ptr2")
                        for j in range(nu):
                            u = 4 * ug + j
                            nc.tensor.transpose(ptt[:, j, :],
                                                v_ln[:, u, 2 * m, 0:64], ident[:])
                            nc.tensor.transpose(ptt2[:, j, :],
                                                v_ln[:, u, 2 * m + 1, 0:64],
                                                ident[:])
                        nc.scalar.activation(
                            lkvT[0:64, 4 * ug:4 * ug + nu, :], ptt[:, :nu, :],
                            AF.Copy)
                        nc.scalar.activation(
                            lkvT[64:128, 4 * ug:4 * ug + nu, :], ptt2[:, :nu, :],
                            AF.Copy)
                    expL = [explp.tile([P, NT, 256], F32R, tag=f"expL{par}",
                                       name=f"expL{par}")
                            for par in range(2)]
                    expG = [expgp.tile([P, 2, 2, 512], F32R, tag=f"expG{par}",
                                       name=f"expG{par}")
                            for par in range(2)]
                    # local sim, both heads interleaved (PE row-group overlap)
                    for u in range(NT):
                        if u == 0:
                            qs, qn = 0, 128
                        elif u == 8:
                            qs, qn = 896, 128
                        else:
                            qs, qn = (u - 1) * 128, 256
                        pls = [plsim.tile([P, 256], F32, tag=f"pls{par}",
                                          name=f"pls{par}")
                               for par in range(2)]
                        for par in range(2):
                            prow = slice(par * 64, par * 64 + 64)
                            nc.tensor.matmul(
                                pls[par][:, :qn], lkvT[prow, u, :],
                                qT[prow, m, qs:qs + qn], start=True, stop=True)
                        for par in range(2):
                            nc.scalar.activation(expL[par][:, u, 0:qn],
                                                 pls[par][:, :qn], AF.Exp)
                            # key block u: SELF for first 128 queries, PREV for rest
                            msk = halom if u == 0 else tri
                            nc.vector.tensor_tensor(
                                out=expL[par][:, u, 0:128],
                                in0=expL[par][:, u, 0:128],
                                in1=msk[:], op=A.mult)
                    # global sim
                    for bb in range(2):
                        for Q in range(2):
                            pgs = [pgsim.tile([P, 512], F32, tag=f"pgs{par}",
                                              name=f"pgs{par}")
                                   for par in range(2)]
                            for par in range(2):
                                prow = slice(par * 64, par * 64 + 64)
                                nc.tensor.matmul(
                                    pgs[par][:], gkvT[prow, bb, m, :],
                                    qT[prow, m, Q * 512:(Q + 1) * 512],
                                    start=True, stop=True)
                            for par in range(2):
                                nc.scalar.activation(
                                    expG[par][:, bb, Q, :], pgs[par][:], AF.Exp)
                    for par in range(2):
                        nc.gpsimd.tensor_tensor(
                            out=expG[par][:], in0=expG[par][:], in1=gmask[:],
                            op=A.mult)
                    # AV + Z accumulation (keys-major)
                    for par in range(2):
                        h = 2 * m + par
                        prow = slice(par * 64, par * 64 + 64)
                        for Q in range(2):
                            avp = pav.tile([65, 512], F32, tag="avp")
                            nc.tensor.matmul(avp[:], gv[:, 0, h, :],
                                             expG[par][:, 0, Q, :],
                                             start=True, stop=False)
                            nc.tensor.matmul(avp[:], gv[:, 1, h, :],
                                             expG[par][:, 1, Q, :],
                                             start=False, stop=False)
                            mm_list = [(0, 0, 128, 0) if Q == 0 else
                                       (4, 128, 128, 0)]
                            for j in range(1, 4):
                                mm_list.append((4 * Q + j, 0, 256, (j - 1) * 128))
                            mm_list.append((4 * Q + 4, 0, 128, 384))
                            for idx, (u, cs, cn, po) in enumerate(mm_list):
                                nc.tensor.matmul(
                                    avp[:, po:po + cn], v_ln[:, u, h, :],
                                    expL[par][:, u, cs:cs + cn],
                                    start=False, stop=(idx == len(mm_list) - 1))
                            # 1/Z = exp(-ln(Z)); broadcast via DRAM round-trip
                            zl = evs.tile([1, 512], F32, tag="zl")
                            nc.scalar.activation(zl[:], avp[64:65, :], AF.Ln)
                            zr = evs.tile([1, 512], F32, tag="zr")
                            nc.scalar.activation(zr[:], zl[:], AF.Exp, scale=-1.0)
                            nc.sync.dma_start(out=zr_dram[h, Q, :], in_=zr[:])
                            zrb = evs.tile([64, 512], F32, tag="zrb")
                            nc.gpsimd.dma_start(
                                out=zrb[:],
                                in_=zr_dram[h, Q, :].unsqueeze(0)
                                .partition_broadcast(64))
                            with nc.allow_low_precision(reason="f32r store"):
                                nc.vector.scalar_tensor_tensor(
                                    out=attnT[prow, m, Q * 512:(Q + 1) * 512],
                                    in0=avp[0:64, :], scalar=1.0, in1=zrb[:],
                                    op0=A.mult, op1=A.mult)

            pool_attn.release()
            pool_qT.release()

            # ---------------- Phase F: final projection ----------------
            with tc.tile_pool(name="wof", bufs=9) as wof_pool, \
                 tc.tile_pool(name="pf", bufs=3, space="PSUM") as pf, \
                 tc.tile_pool(name="outp", bufs=2) as outp:
                wo_k = []
                for k in range(8):
                    wk3 = wof_pool.tile([P, DIM], F32R, tag="wo")
                    nc.sync.dma_start(out=wk3[:], in_=wo[k * P:(k + 1) * P, :])
                    wo_k.append(wk3)
                for tt in range(8):
                    ot = outp.tile([P, DIM], F32, tag="ot")
                    for nh in range(2):
                        psf = pf.tile([P, 512], F32, tag="psf")
                        for m in range(8):
                            nc.tensor.matmul(
                                psf[:], attnT[:, m, tt * P:(tt + 1) * P],
                                wo_k[m][:, nh * 512:(nh + 1) * 512],
                                start=(m == 0),
                                stop=(m == 7 and not nonzero_bo))
                        if nonzero_bo:
                            nc.tensor.matmul(
                                psf[:], ones128[:], bod[:, nh * 512:(nh + 1) * 512],
                                start=False, stop=True)
                        nc.scalar.activation(ot[:, nh * 512:(nh + 1) * 512],
                                             psf[:], AF.Copy)
                    nc.sync.dma_start(out=out_d[tt], in_=ot[:])

            pool_out.release()

    nc.compile()
    return nc


_PROG_CACHE = {}


def _get_program(key):
    if key not in _PROG_CACHE:
        _PROG_CACHE[key] = build_program(*key)
    return _PROG_CACHE[key]


def _host_constants(Wp):
    ident = np.eye(P, dtype=np.float32)
    seg16 = np.zeros((P, 8), np.float32)
    for g in range(8):
        seg16[g * 16:(g + 1) * 16, g] = 1.0
    jk, ii = np.meshgrid(np.arange(P), np.arange(P), indexing="ij")
    tri = (jk <= ii).astype(np.float32)
    onesbig = np.ones((P, NT * 16 * 65), np.float32)
    stats_lhsT = np.zeros((P, 4), np.float32)
    stats_lhsT[0:64, 0] = 1.0 / 64
    stats_lhsT[0:64, 1] = Wp[:, 0]
    stats_lhsT[64:128, 2] = 1.0 / 64
    stats_lhsT[64:128, 3] = Wp[:, 0]
    return ident, seg16, tri, onesbig, stats_lhsT


def kernel(x, Wq, bq, Wkv, bkv, Wp, bp, ln_l_g, ln_l_b, ln_g_g, ln_g_b, Wo, bo):
    # NOTE: bp shifts all segment logits equally (R=1), so the segment softmax
    # is invariant to it; it is deliberately unused.
    x = _f32(x); Wq = _f32(Wq); Wkv = _f32(Wkv); Wo = _f32(Wo)
    bq = _f32(bq); bkv = _f32(bkv); bo = _f32(bo); Wp = _f32(Wp)
    ln_l_g = _f32(ln_l_g); ln_l_b = _f32(ln_l_b)
    ln_g_g = _f32(ln_g_g); ln_g_b = _f32(ln_g_b)

    nontrivial_ln_l = not (np.all(ln_l_g == 1.0) and np.all(ln_l_b == 0.0))
    nontrivial_ln_g = not (np.all(ln_g_g == 1.0) and np.all(ln_g_b == 0.0))
    nonzero_bq = bool(np.any(bq != 0.0))
    nonzero_bkv = bool(np.any(bkv != 0.0))
    nonzero_bo = bool(np.any(bo != 0.0))
    key = (nontrivial_ln_l, nontrivial_ln_g, nonzero_bq, nonzero_bkv, nonzero_bo)
    nc = _get_program(key)

    ident, seg16, tri, onesbig, stats_lhsT = _host_constants(Wp)

    in_maps = []
    for c in range(NC):
        bc, ci = c // 4, c % 4
        tc0 = ci * TOK
        xb = x[bc]
        xtc = np.zeros((DIM, TOKH), np.float32)
        lo = tc0 - HALO
        src_lo = max(lo, 0)
        xtc[:, src_lo - lo:] = xb[src_lo:tc0 + TOK].T
        halom = (np.ones if ci > 0 else np.zeros)((P, P)).astype(np.float32)
        qi = tc0 + np.arange(1024).reshape(2, 512)
        seg = np.arange(256).reshape(2, 128)
        gm = (qi[None, :, None, :] >= (16 * seg[:, None, :, None] + 15))
        gmask = np.ascontiguousarray(
            gm.transpose(2, 0, 1, 3).astype(np.float32))
        im = dict(xt=xtc, wq=Wq, wkv=Wkv, wo=Wo, ident=ident, seg16=seg16,
                  tri=tri, halom=halom, gmask=gmask, stats_lhsT=stats_lhsT,
                  onesbig=onesbig)
        if nontrivial_ln_l:
            im["ln_l_gb"] = np.ascontiguousarray(np.broadcast_to(
                np.stack([ln_l_g, ln_l_b]), (P, 2, 64)).astype(np.float32))
        if nontrivial_ln_g:
            im["ln_g_gb"] = np.ascontiguousarray(np.broadcast_to(
                np.stack([ln_g_g, ln_g_b]), (P, 2, 64)).astype(np.float32))
        if nonzero_bq:
            im["bqs"] = np.ascontiguousarray((bq * SCALE).reshape(8, P).T)
        if nonzero_bkv:
            im["bkvs"] = np.ascontiguousarray(bkv.reshape(8, P).T)
        if nonzero_bo:
            im["bod"] = bo.reshape(1, DIM)
            im["ones128"] = np.ones((1, P), np.float32)
        in_maps.append(im)

    res = run_bass_kernel_spmd(nc, in_maps, list(range(NC)))
    out = np.empty((B, N, DIM), np.float32)
    for c in range(NC):
        bc, ci = c // 4, c % 4
        out[bc, ci * TOK:(ci + 1) * TOK] = res.results[c]["out"].reshape(TOK, DIM)
    return out

